# revision 5
# baseline (speedup 1.0000x reference)
"""NT-Xent contrastive loss kernel for 8 Trainium2 NeuronCores.

Reference computation (N=8192, D=512, tau=0.5):
    zl = l2norm_rows(left); zr = l2norm_rows(right)
    refl    = exp(zl @ zl.T / tau)
    between = exp(zl @ zr.T / tau)
    denom   = refl.sum(1) + between.sum(1) - diag(refl)
    loss    = -log(diag(between) / denom)

Fused per-row form (diag(refl) == e^2 exactly since rows of zl are unit):
    loss[m] = log( S_l[m] + S_r[m] - e^2 ) - 2 * (zl_m . zr_m)
with S_x[m] = sum_n exp(2 * zl_m . zx_n).

Sharding: data-parallel over rows; core c owns rows [c*1024, (c+1)*1024).
The host ships, per core, both tensors in a column-ROLLED chunked K-major
bf16 layout: columns are rotated so the core's own 1024 columns come
first, then split into 4 chunks of 2048 columns shaped [128, 4, 2048]
(partition = k%128, then k//128, then column).  Each chunk is one fully
contiguous 2MB DMA with 16KB per-partition lines.

Per chunk the kernel streams: square (DVE, bf16 2x) -> per-column
sum-of-squares via ones-matmul into a [1,2048] PSUM strip (PE) ->
inv-norm = Exp(-0.5*Ln(ssq)) (ACT, stays on the ln/exp table all kernel)
-> partition_broadcast (GpSimd) -> normalize-multiply into fp8e4 (DVE).
The first chunk's normalized output is the persistent lhsT (own rows,
K-major).  Main similarity matmuls run in fp8 DoubleRow mode (K=256 per
instruction, 0.5 cycles/row) into [128,2048] PSUM tiles; one exp
activation per (m-tile, chunk) consumes the tile with accum_out
producing the partial row-sum.  The between-diagonal is read from the
PSUM of the own-r chunk before exp.  No cross-core communication.
"""

import numpy as np
import ml_dtypes
from contextlib import ExitStack

import concourse.bass as bass
import concourse.tile as tile
from concourse import bacc, mybir
from concourse.bass import ds, ts
from concourse.bass_utils import run_bass_kernel_spmd
from concourse.masks import make_identity

P = 128          # partitions
D = 512          # feature dim
N = 8192         # rows
NCORES = 8
BLK = N // NCORES          # 1024 rows per core
KCH = D // P               # 4 k-chunks of 128
MT = BLK // P              # 8 m-tiles per core
W = 2048                   # columns per streamed chunk
NJ = N // W                # 4 chunks per tensor
NCHUNK = 2 * NJ            # 8 chunks total (l then r)
NG = W // 512              # 4 psum col-groups per chunk
E2 = float(np.exp(2.0))

F32 = mybir.dt.float32
BF16 = mybir.dt.bfloat16
FP8 = mybir.dt.float8e4
AF = mybir.ActivationFunctionType
OP = mybir.AluOpType
DR = mybir.MatmulPerfMode.DoubleRow

_CACHE = {}


def _body(ctx, tc, lch, rch, loss_out):
    nc = tc.nc

    const_pool = ctx.enter_context(tc.tile_pool(name="const", bufs=1))
    persist = ctx.enter_context(tc.tile_pool(name="persist", bufs=1))
    raw_pool = ctx.enter_context(tc.tile_pool(name="raw", bufs=3))
    sq_pool = ctx.enter_context(tc.tile_pool(name="sq", bufs=2))
    nrm_pool = ctx.enter_context(tc.tile_pool(name="nrm", bufs=2))
    zn_pool = ctx.enter_context(tc.tile_pool(name="zn", bufs=2))
    exp_pool = ctx.enter_context(tc.tile_pool(name="expo", bufs=2))
    small = ctx.enter_context(tc.tile_pool(name="small", bufs=2))

    psum = ctx.enter_context(tc.tile_pool(name="ps", bufs=2, space="PSUM"))

    # constants
    ones_col = const_pool.tile([P, 1], BF16, tag="ones_col")
    nc.gpsimd.memset(ones_col[:], 1.0)
    ident = const_pool.tile([P, P], F32, tag="ident")
    make_identity(nc, ident[:])
    neg_e2 = const_pool.tile([P, 1], F32, tag="neg_e2")
    nc.gpsimd.memset(neg_e2[:], -E2)

    # persistent tensors
    zn_own = persist.tile([P, KCH, W], FP8, tag="zn_own")   # chunk 0 of l
    rowsums = persist.tile([P, MT, NCHUNK], F32, tag="rowsums")
    bd = persist.tile([P, MT], F32, tag="bd")

    raws = {}

    def dma_stage(j):
        src = lch[j, :, :, :] if j < NJ else rch[j - NJ, :, :, :]
        raw = raw_pool.tile([P, KCH, W], BF16, tag="raw")
        nc.sync.dma_start(raw[:], src)
        raws[j] = raw

    zns = {}

    def norm_stage(j):
        raw = raws.pop(j)
        sq = sq_pool.tile([P, KCH, W], BF16, tag="sq")
        nc.vector.tensor_mul(sq[:], raw[:], raw[:])

        # Shares the "act" ring slots (same tag) so PSUM stays within 8
        # banks; ring order interleaves [ssq_j+1, act_j_m0..m7, ...].
        ssq = psum.tile([1, W], F32, tag="act")
        for g in range(NG):
            for k in range(KCH):
                nc.tensor.matmul(
                    ssq[:, ds(g * 512, 512)],
                    ones_col[:],
                    sq[:, k, ds(g * 512, 512)],
                    start=(k == 0),
                    stop=(k == KCH - 1),
                )
        lssq = nrm_pool.tile([1, W], F32, tag="lssq")
        nc.scalar.activation(lssq[:], ssq[:], AF.Ln)
        inv1 = nrm_pool.tile([1, W], BF16, tag="inv1")
        nc.scalar.activation(inv1[:], lssq[:], AF.Exp, scale=-0.5)
        invb = nrm_pool.tile([P, W], BF16, tag="invb")
        nc.gpsimd.partition_broadcast(invb[:], inv1[:])

        zn = zn_own if j == 0 else zn_pool.tile([P, KCH, W], FP8, tag="zn")
        for k in range(KCH):
            nc.vector.tensor_mul(zn[:, k, :], raw[:, k, :], invb[:])
        zns[j] = zn

    def main_stage(j):
        zn = zns.pop(j)
        for m in range(MT):
            ps = psum.tile([P, W], F32, tag="act")
            for g in range(NG):
                for i in range(KCH // 2):
                    nc.tensor.matmul(
                        ps[:, ds(g * 512, 512)],
                        zn_own[:, ds(2 * i, 2), ts(m, P)],
                        zn[:, ds(2 * i, 2), ds(g * 512, 512)],
                        start=(i == 0),
                        stop=(i == KCH // 2 - 1),
                        perf_mode=DR,
                    )
            if j == NJ:  # own-r chunk: between-diagonal before exp
                tmp = small.tile([P, P], F32, tag="diagtmp")
                nc.vector.tensor_mul(tmp[:], ps[:, ds(m * P, P)], ident[:])
                nc.vector.tensor_reduce(
                    bd[:, ts(m, 1)], tmp[:], axis=mybir.AxisListType.X, op=OP.add
                )
            eo = exp_pool.tile([P, W], BF16, tag="eo")
            nc.scalar.activation(
                eo[:], ps[:], AF.Exp, scale=2.0,
                accum_out=rowsums[:, m, ds(j, 1)],
            )

    # two-stage software pipeline: norms run one chunk ahead of the main
    # matmul+exp stage so the ACT engine never waits on the norm chain.
    dma_stage(0)
    dma_stage(1)
    norm_stage(0)
    for j in range(NCHUNK):
        if j + 2 < NCHUNK:
            dma_stage(j + 2)
        if j + 1 < NCHUNK:
            norm_stage(j + 1)
        main_stage(j)

    # ---- loss epilogue -----------------------------------------------------
    s_all = small.tile([P, MT], F32, tag="s_all")
    nc.vector.tensor_reduce(
        s_all[:], rowsums[:], axis=mybir.AxisListType.X, op=OP.add
    )
    logd = small.tile([P, MT], F32, tag="logd")
    nc.scalar.activation(logd[:], s_all[:], AF.Ln, bias=neg_e2[:])
    loss_sb = small.tile([P, MT], F32, tag="loss_sb")
    nc.vector.scalar_tensor_tensor(
        out=loss_sb[:], in0=bd[:], scalar=-2.0, in1=logd[:],
        op0=OP.mult, op1=OP.add,
    )
    nc.sync.dma_start(loss_out[:, :], loss_sb[:])


class _pin_act_table:
    """During compile, present activation tables where Exp/Ln appear ONLY in
    the combined natural_log_exp table, so the table-load pass emits a single
    hoisted load instead of ping-ponging between an exp-table and an
    ln-table (1.3us per switch).  Table order/indices are preserved; only
    the membership sets are filtered.  Restored immediately after compile."""

    COMBINED = "natural_log_exp_and_others"

    def __enter__(self):
        import concourse.bacc as bacc_mod
        self._mod = bacc_mod
        self._orig = bacc_mod.get_activation_tables

        orig = self._orig
        combined = self.COMBINED

        def patched(arch):
            tabs = orig(arch)
            if combined not in tabs:
                return tabs
            pin = {AF.Exp, AF.Ln}
            out = {}
            for name, s in tabs.items():
                out[name] = set(s) if name == combined else set(s) - pin
            return out

        bacc_mod.get_activation_tables = patched
        return self

    def __exit__(self, *exc):
        self._mod.get_activation_tables = self._orig
        return False


def _build():
    nc = bacc.Bacc("TRN2", target_bir_lowering=False, debug=False, num_devices=NCORES)
    lch = nc.dram_tensor("lch", [NJ, P, KCH, W], BF16, kind="ExternalInput").ap()
    rch = nc.dram_tensor("rch", [NJ, P, KCH, W], BF16, kind="ExternalInput").ap()
    loss = nc.dram_tensor("loss", [P, MT], F32, kind="ExternalOutput").ap()
    with tile.TileContext(nc) as tc, ExitStack() as ctx:
        _body(ctx, tc, lch, rch, loss)
    with _pin_act_table():
        nc.compile()
    return nc


def _get_nc():
    if "nc" not in _CACHE:
        _CACHE["nc"] = _build()
    return _CACHE["nc"]


def _chunked(xT, c):
    """xT: [KCH, P, N] bf16 K-major. Returns [NJ, P, KCH, W] rolled so core
    c's own columns come first."""
    r = np.roll(xT, -c * BLK, axis=2)
    # [KCH, P, N] -> [NJ, P, KCH, W]
    out = np.empty((NJ, P, KCH, W), dtype=xT.dtype)
    for j in range(NJ):
        out[j] = r[:, :, j * W:(j + 1) * W].transpose(1, 0, 2)
    return out


def _in_maps(left, right):
    bf = ml_dtypes.bfloat16
    left = np.asarray(left, dtype=np.float32)
    right = np.asarray(right, dtype=np.float32)
    lT = np.ascontiguousarray(left.T).astype(bf).reshape(KCH, P, N)
    rT = np.ascontiguousarray(right.T).astype(bf).reshape(KCH, P, N)
    maps = []
    for c in range(NCORES):
        maps.append({
            "lch": _chunked(lT, c),
            "rch": _chunked(rT, c),
        })
    return maps


def _gather(results):
    # loss dram tile is [128 partitions, 8 m-tiles]; row m = t*128 + p
    parts = [np.asarray(r["loss"]).T.reshape(-1) for r in results]
    return np.concatenate(parts).astype(np.float32)


def run_traced(left, right):
    """Run with NTFF profiling; returns (loss, exec_time_ns)."""
    res = run_bass_kernel_spmd(
        _get_nc(), _in_maps(left, right), list(range(NCORES)), trace=True
    )
    return _gather(res.results), res.exec_time_ns


def kernel(left, right):
    res = run_bass_kernel_spmd(
        _get_nc(), _in_maps(left, right), list(range(NCORES))
    )
    return _gather(res.results)
